# revision 7
# baseline (speedup 1.0000x reference)
"""Trainium2 Bass kernel: single-head causal attention (v3.1).

Problem: x[4,2048,1024] f32; q/k/v = x@W* + b* (head dim 128);
out = softmax(causal(q k^T / sqrt(128))) @ v.

Sharding: 8 cores = 4 batches x 2 causal "wedges". Within a batch, the 16
query blocks (128 rows each) are interleaved between the two cores
(h=0 takes odd global blocks, h=1 even) so both cores carry an identical
static schedule: slot p attends exactly L_p = 2p+2 local key blocks.
Per-core key order is a host-side permutation (h=0 identity, h=1
adjacent-pair swap) that puts slot p's own (diagonal) block at local
position 2p+1; the wedge difference is carried by a mask input, so a
single NEFF serves all 8 cores (SPMD).

v3.1 design notes (trace-driven):
  - DMA engines round-robin FAIRLY across all in-flight transfers, so
    issue order does not give priority. Transfers are therefore CHAINED:
    each later DMA's issue is gated on the previous transfer's completion
    via a 2-element gpsimd dummy copy that reads the previous tile and
    writes the next tile (WAW dep delays the dma_start). Order:
    consts+x8 chunk0 free-running, then x8 c1->c2->c3->xg0->xg1->xg2->xg3.
  - x8 is shipped KEY-CHUNK-major (4 chunks x 512 keys, each with all
    1024 contraction rows) so q/k projections complete per-chunk and the
    S^T/exp chain starts ~15us instead of ~20us. Chunks 0,1 are the
    own-query columns (feed qT and kT positions 0..7), chunks 2,3 feed
    kT positions 8..15.
  - kT is stored as four independent [128,512] tiles (kt0..kt3) and qT as
    two, each written by exactly one psum copy: dependency tracking is
    whole-tile, so shared tiles would stall early S^T on late copies.
  - PE pstate ramps to full clock only after ~3us of CONTINUOUS work:
    warmup matmuls bridge until x8 chunk0 lands, no gaps.
  - q/k projections in fp8 e4m3 DoubleRow (contraction 256/matmul);
    weights prescaled x8, 1/8 folded into the psum->sbuf copy scale.
    ACT copies only kt0 (first-needed); DVE does qT_lo/qT_hi/kt1/kt2/kt3
    (ACT must stay free: the 16-exp chain is the serial mid-phase).
  - S^T per key block j into a 2-bank [128,1024] psum tile (pieces split
    at the bank boundary), ONE exp per block.
  - v projection direct in [key, dk] orientation (lhsT = x^T key-block,
    rhs = Wv chunk): no transposes. bv is folded into the output epilogue
    (softmax weights sum to 1): out = o_ps*rcp + bv via one
    scalar_tensor_tensor. v_aug is per-group tiles (whole-tile deps).
  - PV bursts (denominator via the v_aug ones-column); bursts 6,7 split
    so only blocks 12..15 + finalize remain after xg group 3 lands.
  - PSUM: kq uses per-chunk [128,512] tiles (kps x2 + qps x2 bufs,
    4 banks) concurrent with spool 2x[128,1024] (4 banks); opool(3)+
    vpool(1) reuse the kq banks after release.
"""

import numpy as np

B, T, D, DK = 4, 2048, 1024, 128
NBLK = T // 128      # 16 key blocks per core
NSLOT = 8            # q slots per core (NSLOT*128 = 1024 q rows)
NCHUNK = D // 128    # bf16 m-chunks (v projection)
NDC = D // 256       # fp8 double-chunks (q/k projections)
NG = 4               # v key groups (512 keys each)
GW = T // NG         # group width (512)
NKC = 4              # x8 key chunks (512 keys each)
KCW = T // NKC
SCALE = 1.0 / np.sqrt(np.float32(DK))
WS = 8.0             # fp8 weight prescale (power of 2; undone in psum copy)
WARMUP_MMS = 5

_built = None


def _build():
    from contextlib import ExitStack

    import concourse.mybir as mybir
    import concourse.tile as tile
    from concourse import bacc

    f32 = mybir.dt.float32
    bf16 = mybir.dt.bfloat16
    fp8 = mybir.dt.float8e4
    Act = mybir.ActivationFunctionType
    Alu = mybir.AluOpType
    DR = mybir.MatmulPerfMode.DoubleRow

    nc = bacc.Bacc("TRN2", target_bir_lowering=False, debug=False, num_devices=8)

    # all bulk inputs partition-major: [128, ...] with >=4KB contiguous runs
    x8p = nc.dram_tensor("x8p", [128, NKC * NDC * 2 * KCW], fp8,
                         kind="ExternalInput").ap()
    xgp = nc.dram_tensor("xgp", [128, NG * NCHUNK * GW], bf16,
                         kind="ExternalInput").ap()
    # packed consts: fp8 = wk8|wq8; f32 = [bq, bk*SCALE, pad, bvv(128)];
    # bf16 = wv | masks
    cstf8 = nc.dram_tensor("cstf8", [128, 2 * NDC * 2 * DK], fp8,
                           kind="ExternalInput").ap()
    cst32 = nc.dram_tensor("cst32", [128, 3 + DK], f32, kind="ExternalInput").ap()
    cst16 = nc.dram_tensor("cst16", [128, NCHUNK * DK + 256], bf16,
                           kind="ExternalInput").ap()
    o = nc.dram_tensor("o", [NSLOT * 128, DK], f32, kind="ExternalOutput").ap()

    with tile.TileContext(nc) as tc, ExitStack() as ctx:
        const = ctx.enter_context(tc.tile_pool(name="const", bufs=1))
        sbufs = ctx.enter_context(tc.tile_pool(name="sbufs", bufs=1))
        x8_pool = ctx.enter_context(tc.tile_pool(name="x8_pool", bufs=NKC))
        xg_pool = ctx.enter_context(tc.tile_pool(name="xg_pool", bufs=NG))
        out_pool = ctx.enter_context(tc.tile_pool(name="out_pool", bufs=3))

        # ---- DMA wave 1 (free-running): consts + x8 chunk 0
        cstf8_sb = const.tile([128, 2, NDC, 2, DK], fp8, tag="cstf8")
        nc.sync.dma_start(out=cstf8_sb, in_=cstf8)

        x8s = [x8_pool.tile([128, NDC, 2, KCW], fp8, tag="x8", name=f"x8_{t}")
               for t in range(NKC)]

        def load_x8(t):
            nc.sync.dma_start(
                out=x8s[t],
                in_=x8p[:, NDC * 2 * KCW * t : NDC * 2 * KCW * (t + 1)],
            )

        load_x8(0)
        cst32_sb = const.tile([128, 3 + DK], f32, tag="cst32")
        nc.sync.dma_start(out=cst32_sb, in_=cst32)
        cst16_sb = const.tile([128, NCHUNK * DK + 256], bf16, tag="cst16")
        nc.sync.dma_start(out=cst16_sb, in_=cst16)

        xgs = [xg_pool.tile([128, NCHUNK, GW], bf16, tag="xg", name=f"xg{g}")
               for g in range(NG)]

        # ---- DMA chain: each transfer's issue gated on the previous one's
        # completion via a tiny gpsimd copy (reads prev tile -> WAW-gates the
        # next dma_start). Defeats the fair round-robin so the streams arrive
        # in priority order: x8 c1, c2, c3, then xg g0..g3.
        for t in range(1, NKC):
            nc.gpsimd.tensor_copy(
                x8s[t][:, 0, 0, 0:2], x8s[t - 1][:, 0, 0, 0:2]
            )
            load_x8(t)
        for g in range(NG):
            prev = x8s[NKC - 1][:, 0, 0, 0:2] if g == 0 else xgs[g - 1][:, 0, 0:2]
            nc.gpsimd.tensor_copy(xgs[g][:, 0, 0:2], prev)
            nc.sync.dma_start(
                out=xgs[g], in_=xgp[:, NCHUNK * GW * g : NCHUNK * GW * (g + 1)]
            )

        wk8_sb = cstf8_sb[:, 0]
        wq8_sb = cstf8_sb[:, 1]
        bq_sb = cst32_sb[:, 0:1]
        bks_sb = cst32_sb[:, 1:2]
        bvv_sb = cst32_sb[:, 3 : 3 + DK]
        wv_sb = cst16_sb[:, 0 : NCHUNK * DK]
        mask_sb = cst16_sb[:, NCHUNK * DK :]

        # ---- PE warmup: continuous PE activity from t0 until x8 chunk 0
        # lands (pstate ramps to full clock after ~3us of uninterrupted
        # work) + pulls the exp ACT_TABLE_LOAD early.
        with tc.tile_pool(name="warmps", bufs=1, space="PSUM") as warmps:
            wsrc = sbufs.tile([128, 512], bf16, tag="wsrc")
            nc.vector.memset(wsrc, 0.0)
            wdst = warmps.tile([128, 512], f32, tag="warm")
            for _ in range(WARMUP_MMS):
                nc.tensor.matmul(
                    wdst, lhsT=wsrc[:, 0:128], rhs=wsrc, start=True, stop=True
                )
            wexp = sbufs.tile([128, 1], f32, tag="wexp")
            nc.scalar.activation(out=wexp, in_=wsrc[:, 0:1], func=Act.Exp, scale=1.0)

        # ---- q/k projections, key-chunk-major (fp8 DoubleRow).
        # kT positions 0..7 = own-query columns (x8 chunks 0,1 = qT source),
        # positions 8..15 = chunks 2,3. Each 512-wide psum accumulation
        # completes right after its chunk arrives; per-piece sbuf tiles.
        kts = [sbufs.tile([128, 512], bf16, tag=f"kt{i}", name=f"kt{i}")
               for i in range(4)]
        qT_lo = sbufs.tile([128, 512], bf16, tag="qTl")     # slots 0..3
        qT_hi = sbufs.tile([128, 512], bf16, tag="qTh")     # slots 4..7

        spool = tc.alloc_tile_pool(name="spool", bufs=2, space="PSUM")
        kqpool = tc.alloc_tile_pool(name="kqpool", bufs=2, space="PSUM")

        def kq_mms(dst_ps, w_sb, t):
            for dc in range(NDC):
                nc.tensor.matmul(
                    dst_ps,
                    lhsT=w_sb[:, dc, :, :],
                    rhs=x8s[t][:, dc],
                    start=(dc == 0),
                    stop=(dc == NDC - 1),
                    perf_mode=DR,
                )

        def dve_copy(dst, src_ps, scale, bias):
            nc.vector.tensor_scalar(
                out=dst, in0=src_ps, scalar1=float(scale), scalar2=bias,
                op0=Alu.mult, op1=Alu.add,
            )

        kps = [None] * NKC
        qps = [None] * 2
        # chunk 0: kT piece 0 first (ACT copy), then qT_lo
        kps[0] = kqpool.tile([128, 512], f32, tag="kps", name="kps0")
        kq_mms(kps[0], wk8_sb, 0)
        qps[0] = kqpool.tile([128, 512], f32, tag="qps", name="qps0")
        kq_mms(qps[0], wq8_sb, 0)
        nc.scalar.activation(
            out=kts[0], in_=kps[0], func=Act.Identity, bias=bks_sb,
            scale=SCALE / WS,
        )
        dve_copy(qT_lo, qps[0], 1.0 / WS, bq_sb)
        # chunk 1: qT_hi first (unblocks every exp), then kT piece 1
        qps[1] = kqpool.tile([128, 512], f32, tag="qps", name="qps1")
        kq_mms(qps[1], wq8_sb, 1)
        kps[1] = kqpool.tile([128, 512], f32, tag="kps", name="kps1")
        kq_mms(kps[1], wk8_sb, 1)
        dve_copy(qT_hi, qps[1], 1.0 / WS, bq_sb)
        dve_copy(kts[1], kps[1], SCALE / WS, bks_sb)
        # chunks 2,3: kT pieces 2,3 (psum slots recycled)
        for t in (2, 3):
            kps[t] = kqpool.tile([128, 512], f32, tag="kps", name=f"kps{t}")
            kq_mms(kps[t], wk8_sb, t)
            dve_copy(kts[t], kps[t], SCALE / WS, bks_sb)
        kqpool.release()

        # ---- attention: S^T/exp, v groups, PV bursts ----
        pt_pool = ctx.enter_context(tc.tile_pool(name="pt_pool", bufs=NBLK))
        vpool = tc.alloc_tile_pool(name="vpool", bufs=1, space="PSUM")
        opool = tc.alloc_tile_pool(name="opool", bufs=3, space="PSUM")

        pts = [None] * NBLK
        v_augs = [None] * NG
        o_pss = [None] * NSLOT

        def kpos(j):
            # column position of local key block j in the reordered x8/kT
            return (j - 1) // 2 if j % 2 == 1 else NSLOT + j // 2

        def emit_st(j):
            """S^T for key block j into a 2-bank psum tile, then one exp
            over the whole active range, then the frontier mask multiply."""
            sj = j // 2           # first active slot for this key position
            q0 = 128 * sj
            qn = NSLOT * 128 - q0
            pt = pt_pool.tile([128, qn], bf16, tag="pt", name=f"pt{j}")
            pts[j] = pt
            kp = kpos(j)
            kt = kts[kp // 4]
            kp = kp % 4
            s_ps = spool.tile([128, 1024], f32, tag="st", name=f"s{j}")
            # matmul pieces split at the qT_lo/qT_hi boundary (col 512),
            # which is also the psum bank boundary
            if q0 < 512:
                pieces = [(qT_lo, q0, q0, 512 - q0), (qT_hi, 0, 512, 512)]
            else:
                pieces = [(qT_hi, q0 - 512, q0, 1024 - q0)]
            for qtile, qoff, doff, sz in pieces:
                nc.tensor.matmul(
                    s_ps[:, doff : doff + sz],
                    lhsT=kt[:, 128 * kp : 128 * kp + 128],
                    rhs=qtile[:, qoff : qoff + sz],
                    start=True,
                    stop=True,
                )
            nc.scalar.activation(
                out=pt, in_=s_ps[:, q0:1024], func=Act.Exp, scale=1.0,
            )
            # mask the frontier slot multiplicatively (exp(s+m) = exp(s)*m01):
            # even j -> maskA (wedge-dependent), odd j -> maskB (causal tri)
            sel = j % 2
            nc.vector.tensor_mul(
                pt[:, 0:128],
                pt[:, 0:128],
                mask_sb[:, 128 * sel : 128 * (sel + 1)],
            )

        def emit_vgroup(g):
            """v for key blocks 4g..4g+3, directly in [key, dk] orientation.
            lhsT = x^T key-block (128 keys), rhs = Wv chunk; accumulate over
            the 8 contraction chunks; no bias (folded into the epilogue)."""
            v_aug = const.tile([128, 4, DK + 1], bf16, tag=f"vaug{g}",
                               name=f"vaug{g}")
            v_augs[g] = v_aug
            nc.vector.memset(v_aug[:, :, DK : DK + 1], 1.0)
            vg_ps = vpool.tile([128, 4, DK], f32, tag="vps", name=f"vg{g}")
            for b in range(4):
                for c in range(NCHUNK):
                    nc.tensor.matmul(
                        vg_ps[:, b, :],
                        lhsT=xgs[g][:, c, 128 * b : 128 * (b + 1)],
                        rhs=wv_sb[:, 128 * c : 128 * (c + 1)],
                        start=(c == 0),
                        stop=(c == NCHUNK - 1),
                    )
            nc.vector.tensor_copy(v_aug[:, :, 0:DK], vg_ps)

        def emit_burst(p, j_lo, j_hi):
            """PV accumulation for slot p over key blocks j_lo..j_hi-1."""
            if j_lo == 0:
                o_pss[p] = opool.tile([128, DK + 1], f32, tag="o",
                                      name=f"o_ps{p}")
            o_ps = o_pss[p]
            for jj in range(j_lo, j_hi):
                nc.tensor.matmul(
                    o_ps,
                    lhsT=pts[jj][:, 128 * (p - jj // 2) : 128 * (p - jj // 2) + 128],
                    rhs=v_augs[jj // 4][:, jj % 4, :],
                    start=(jj == 0),
                    stop=(jj == 2 * p + 1),
                )

        def emit_finish(p):
            """out = o_ps * (1/den) + bv, then DMA out."""
            o_ps = o_pss[p]
            rcp = out_pool.tile([128, 1], f32, tag="rcp")
            nc.vector.reciprocal(rcp, o_ps[:, DK : DK + 1])
            ob = out_pool.tile([128, DK], f32, tag="ob")
            nc.vector.scalar_tensor_tensor(
                out=ob, in0=o_ps[:, 0:DK], scalar=rcp, in1=bvv_sb,
                op0=Alu.mult, op1=Alu.add,
            )
            nc.sync.dma_start(out=o[128 * p : 128 * (p + 1), :], in_=ob)

        def full_burst(p):
            emit_burst(p, 0, 2 * p + 2)
            emit_finish(p)

        # emission order = exp order on ACT (the serial resource) with kt
        # readiness (kt0 ~14, kt1 ~15.7, kt2 ~16.5, kt3 ~17.4) and PV-burst
        # unlocking in mind; v groups and bursts fill the PE between S^Ts.
        emit_st(1); emit_st(3); emit_st(5); emit_st(7)
        emit_st(0); emit_st(2)
        emit_vgroup(0)
        emit_st(9); emit_st(11)
        full_burst(0)
        full_burst(1)
        emit_st(4); emit_st(6)
        emit_vgroup(1)
        full_burst(2)
        full_burst(3)
        emit_st(8); emit_st(10)
        emit_vgroup(2)
        emit_st(13); emit_st(15)
        full_burst(4)
        full_burst(5)
        emit_st(12); emit_st(14)
        emit_burst(6, 0, 12)
        emit_burst(7, 0, 12)
        emit_vgroup(3)
        emit_burst(6, 12, 14)
        emit_finish(6)
        emit_burst(7, 12, 16)
        emit_finish(7)

        opool.release()
        vpool.release()
        spool.release()

    nc.compile()
    return nc


def get_built():
    global _built
    if _built is None:
        _built = _build()
    return _built


def _pos2glob(h):
    if h == 0:
        return list(range(NBLK))
    return [j + 1 if j % 2 == 0 else j - 1 for j in range(NBLK)]


def _pack_w_bf16(W):
    """[D, DK] -> [128, NCHUNK*DK] with column block c holding rows 128c..."""
    import ml_dtypes
    return np.ascontiguousarray(
        np.asarray(W, np.float32).reshape(NCHUNK, 128, DK).transpose(1, 0, 2)
        .reshape(128, NCHUNK * DK).astype(ml_dtypes.bfloat16)
    )


def _pack_w_fp8(W):
    """[D, DK] -> [128, NDC*2*DK] e4m3: [p, ((dc*2+i)*DK)+d] = e4m3(WS*W[256dc+128i+p, d])."""
    import ml_dtypes
    Ws = np.asarray(W, np.float32) * WS
    return np.ascontiguousarray(
        Ws.reshape(NDC, 2, 128, DK).transpose(2, 0, 1, 3)
        .reshape(128, NDC * 2 * DK).astype(ml_dtypes.float8_e4m3)
    )


def make_in_map(x_b, Wq, bq, Wk, bk, Wv, bv, h, xT_pre=None, x8T_pre=None):
    """Build one core's input dict. x_b: [T, D] fp32 for this core's batch.
    xT_pre/x8T_pre: optional precomputed transposed/cast copies (shared by
    both wedge cores of a batch; h=0 uses as-is, h=1 column-permutes)."""
    import ml_dtypes
    bf = ml_dtypes.bfloat16
    if xT_pre is None:
        xT_pre = np.ascontiguousarray(x_b.T.astype(bf))
    if x8T_pre is None:
        x8T_pre = np.ascontiguousarray(x_b.T.astype(ml_dtypes.float8_e4m3))
    if h == 0:
        xT_loc, x8T_loc = xT_pre, x8T_pre
    else:
        p2g = _pos2glob(h)
        cols = np.concatenate([np.arange(128 * g, 128 * (g + 1)) for g in p2g])
        xT_loc = np.ascontiguousarray(xT_pre[:, cols])
        x8T_loc = np.ascontiguousarray(x8T_pre[:, cols])
    # x8 column order: own-query blocks (odd locals, slot order) first, then
    # the even locals -- chunks 0,1 feed the q projection and kT pos 0..7
    korder = list(range(1, NBLK, 2)) + list(range(0, NBLK, 2))
    qcols = np.concatenate([np.arange(128 * j, 128 * (j + 1)) for j in korder])
    # x8p[p, ((t*NDC+dc)*2+i)*KCW + c] = x8T[256dc+128i+p, qcols[512t+c]]
    x8q = x8T_loc[:, qcols]                       # [1024 rows d, 2048 cols]
    x8p = np.ascontiguousarray(
        x8q.reshape(NDC, 2, 128, NKC, KCW).transpose(2, 3, 0, 1, 4)
        .reshape(128, NKC * NDC * 2 * KCW)
    )
    # xgp[p, (g*NCHUNK+c)*GW + t'] = xT[128c+p, GW*g+t']  (partition-major)
    xgp = np.ascontiguousarray(
        xT_loc.reshape(NCHUNK, 128, NG, GW).transpose(1, 2, 0, 3)
        .reshape(128, NG * NCHUNK * GW)
    )
    maskA = (np.ones if h == 0 else np.zeros)((128, 128), bf)
    kk = np.arange(128)
    maskB = np.where(kk[:, None] <= kk[None, :], 1.0, 0.0).astype(bf)
    # cst32: [bq, bk*SCALE, pad, bvv(128 cols, bv broadcast to all parts)]
    cst32 = np.zeros((128, 3 + DK), np.float32)
    cst32[:, 0] = np.asarray(bq, np.float32)
    cst32[:, 1] = np.asarray(bk, np.float32) * SCALE
    cst32[:, 3:] = np.asarray(bv, np.float32)[None, :]
    cstf8 = np.concatenate([_pack_w_fp8(Wk), _pack_w_fp8(Wq)], axis=1)
    cst16 = np.concatenate([_pack_w_bf16(Wv), maskA, maskB], axis=1)
    return {
        "x8p": x8p,
        "xgp": xgp,
        "cstf8": np.ascontiguousarray(cstf8),
        "cst32": np.ascontiguousarray(cst32),
        "cst16": np.ascontiguousarray(cst16),
    }


def gather_out(results):
    """results: list of 8 dicts with 'o' [1024, 128] -> full [B, T, DK]."""
    out = np.zeros((B, T, DK), np.float32)
    for core in range(8):
        b, h = core // 2, core % 2
        ob = results[core]["o"]
        for p in range(NSLOT):
            g = 2 * p + 1 - h
            out[b, 128 * g : 128 * (g + 1), :] = ob[128 * p : 128 * (p + 1), :]
    return out


def kernel(x, Wq, bq, Wk, bk, Wv, bv):
    import ml_dtypes
    from concourse.bass_utils import run_bass_kernel_spmd

    x = np.asarray(x, np.float32)
    args = [np.asarray(a, np.float32) for a in (Wq, bq, Wk, bk, Wv, bv)]
    nc = get_built()
    # one transpose+cast per batch, shared by its two wedge cores
    xT_pres = [np.ascontiguousarray(x[b].T.astype(ml_dtypes.bfloat16))
               for b in range(B)]
    x8T_pres = [np.ascontiguousarray(x[b].T.astype(ml_dtypes.float8_e4m3))
                for b in range(B)]
    in_maps = [
        make_in_map(x[core // 2], args[0], args[1], args[2], args[3], args[4],
                    args[5], core % 2, xT_pre=xT_pres[core // 2],
                    x8T_pre=x8T_pres[core // 2])
        for core in range(8)
    ]
    res = run_bass_kernel_spmd(nc, in_maps, core_ids=list(range(8)))
    return gather_out(res.results)


if __name__ == "__main__":
    rng = np.random.default_rng(0)
    x = rng.standard_normal((B, T, D), dtype=np.float32)
    Wq = rng.standard_normal((D, DK), dtype=np.float32) * 0.03
    out = kernel(x, Wq, np.zeros(DK, np.float32), Wq, np.zeros(DK, np.float32),
                 Wq, np.zeros(DK, np.float32))
    print(out.shape)


# revision 9
# speedup vs baseline: 1.3391x; 1.3391x over previous
"""Trainium2 Bass kernel: single-head causal attention (v3.1).

Problem: x[4,2048,1024] f32; q/k/v = x@W* + b* (head dim 128);
out = softmax(causal(q k^T / sqrt(128))) @ v.

Sharding: 8 cores = 4 batches x 2 causal "wedges". Within a batch, the 16
query blocks (128 rows each) are interleaved between the two cores
(h=0 takes odd global blocks, h=1 even) so both cores carry an identical
static schedule: slot p attends exactly L_p = 2p+2 local key blocks.
Per-core key order is a host-side permutation (h=0 identity, h=1
adjacent-pair swap) that puts slot p's own (diagonal) block at local
position 2p+1; the wedge difference is carried by a mask input, so a
single NEFF serves all 8 cores (SPMD).

v3.1 design notes (trace-driven):
  - DMA engines round-robin FAIRLY across all in-flight transfers, so
    issue order does not give priority. Transfers are therefore CHAINED:
    each later DMA's issue is gated on the previous transfer's completion
    via a 2-element gpsimd dummy copy that reads the previous tile and
    writes the next tile (WAW dep delays the dma_start). Order:
    consts+x8 chunk0 free-running, then x8 c1->c2->c3->xg0->xg1->xg2->xg3.
  - x8 is shipped KEY-CHUNK-major (4 chunks x 512 keys, each with all
    1024 contraction rows) so q/k projections complete per-chunk and the
    S^T/exp chain starts ~15us instead of ~20us. Chunks 0,1 are the
    own-query columns (feed qT and kT positions 0..7), chunks 2,3 feed
    kT positions 8..15.
  - kT is stored as four independent [128,512] tiles (kt0..kt3) and qT as
    two, each written by exactly one psum copy: dependency tracking is
    whole-tile, so shared tiles would stall early S^T on late copies.
  - PE pstate ramps to full clock only after ~3us of CONTINUOUS work:
    warmup matmuls bridge until x8 chunk0 lands, no gaps.
  - q/k projections in fp8 e4m3 DoubleRow (contraction 256/matmul);
    weights prescaled x8, 1/8 folded into the psum->sbuf copy scale.
    ACT copies only kt0 (first-needed); DVE does qT_lo/qT_hi/kt1/kt2/kt3
    (ACT must stay free: the 16-exp chain is the serial mid-phase).
  - S^T per key block j into a 2-bank [128,1024] psum tile (pieces split
    at the bank boundary), ONE exp per block.
  - v projection direct in [key, dk] orientation (lhsT = x^T key-block,
    rhs = Wv chunk): no transposes. bv is folded into the output epilogue
    (softmax weights sum to 1): out = o_ps*rcp + bv via one
    scalar_tensor_tensor. v_aug is per-group tiles (whole-tile deps).
  - PV bursts (denominator via the v_aug ones-column); bursts 6,7 split
    so only blocks 12..15 + finalize remain after xg group 3 lands.
  - PSUM: kq uses per-chunk [128,512] tiles (kps x2 + qps x2 bufs,
    4 banks) concurrent with spool 2x[128,1024] (4 banks); opool(3)+
    vpool(1) reuse the kq banks after release.
"""

import numpy as np

B, T, D, DK = 4, 2048, 1024, 128
NBLK = T // 128      # 16 key blocks per core
NSLOT = 8            # q slots per core (NSLOT*128 = 1024 q rows)
NCHUNK = D // 128    # bf16 m-chunks (v projection)
NDC = D // 256       # fp8 double-chunks (q/k projections)
NG = 4               # v key groups (512 keys each)
GW = T // NG         # group width (512)
NKC = 4              # x8 key chunks (512 keys each)
KCW = T // NKC
SCALE = 1.0 / np.sqrt(np.float32(DK))
WS = 8.0             # fp8 weight prescale (power of 2; undone in psum copy)
WARMUP_MMS = 7

_built = None


def _build():
    from contextlib import ExitStack

    import concourse.mybir as mybir
    import concourse.tile as tile
    from concourse import bacc

    f32 = mybir.dt.float32
    bf16 = mybir.dt.bfloat16
    fp8 = mybir.dt.float8e4
    Act = mybir.ActivationFunctionType
    Alu = mybir.AluOpType
    DR = mybir.MatmulPerfMode.DoubleRow

    nc = bacc.Bacc("TRN2", target_bir_lowering=False, debug=False, num_devices=8)

    # all bulk inputs partition-major: [128, ...] with >=4KB contiguous runs
    x8p = nc.dram_tensor("x8p", [128, NKC * NDC * 2 * KCW], fp8,
                         kind="ExternalInput").ap()
    xgp = nc.dram_tensor("xgp", [128, NG * NCHUNK * GW], bf16,
                         kind="ExternalInput").ap()
    # packed consts: fp8 = wk8|wq8; f32 = [bq, bk*SCALE, pad, bvv(128)];
    # bf16 = wv | masks
    cstf8 = nc.dram_tensor("cstf8", [128, 2 * NDC * 2 * DK], fp8,
                           kind="ExternalInput").ap()
    cst32 = nc.dram_tensor("cst32", [128, 3 + DK], f32, kind="ExternalInput").ap()
    cst16 = nc.dram_tensor("cst16", [128, NCHUNK * DK + 256], bf16,
                           kind="ExternalInput").ap()
    o = nc.dram_tensor("o", [NSLOT * 128, DK], f32, kind="ExternalOutput").ap()

    with tile.TileContext(nc) as tc, ExitStack() as ctx:
        const = ctx.enter_context(tc.tile_pool(name="const", bufs=1))
        sbufs = ctx.enter_context(tc.tile_pool(name="sbufs", bufs=1))
        x8_pool = ctx.enter_context(tc.tile_pool(name="x8_pool", bufs=NKC))
        xg_pool = ctx.enter_context(tc.tile_pool(name="xg_pool", bufs=NG))
        out_pool = ctx.enter_context(tc.tile_pool(name="out_pool", bufs=3))

        # ---- all input DMAs up-front, free-running. The DMA engines
        # round-robin FAIRLY across in-flight transfers, so bandwidth share
        # is proportional to transfer COUNT and staggering comes from issue
        # order (~0.6us per issue on the sync queue). x8 is split into 8
        # sub-transfers so it dominates the early wire; xg groups get
        # (3,2,2,1) sub-transfers so they finish progressively.
        cstf8_sb = const.tile([128, 2, NDC, 2, DK], fp8, tag="cstf8")
        nc.sync.dma_start(out=cstf8_sb, in_=cstf8)
        cst32_sb = const.tile([128, 3 + DK], f32, tag="cst32")
        nc.sync.dma_start(out=cst32_sb, in_=cst32)

        x8s = [x8_pool.tile([128, NDC, 2, KCW], fp8, tag="x8", name=f"x8_{t}")
               for t in range(NKC)]
        for t in range(NKC):
            base = NDC * 2 * KCW * t
            half = NDC * KCW  # 2 of 4 dc double-chunks
            nc.sync.dma_start(
                out=x8s[t][:, 0:2], in_=x8p[:, base : base + half]
            )
            nc.sync.dma_start(
                out=x8s[t][:, 2:4], in_=x8p[:, base + half : base + 2 * half]
            )

        cst16_sb = const.tile([128, NCHUNK * DK + 256], bf16, tag="cst16")
        nc.sync.dma_start(out=cst16_sb, in_=cst16)

        xgs = [xg_pool.tile([128, NCHUNK, GW], bf16, tag="xg", name=f"xg{g}")
               for g in range(NG)]
        xg_splits = [(3, 3, 2), (4, 4), (4, 4), (8,)]
        for g in range(NG):
            c0 = 0
            for nsub in xg_splits[g]:
                nc.sync.dma_start(
                    out=xgs[g][:, c0 : c0 + nsub],
                    in_=xgp[:, (g * NCHUNK + c0) * GW : (g * NCHUNK + c0 + nsub) * GW],
                )
                c0 += nsub

        wk8_sb = cstf8_sb[:, 0]
        wq8_sb = cstf8_sb[:, 1]
        bq_sb = cst32_sb[:, 0:1]
        bks_sb = cst32_sb[:, 1:2]
        bvv_sb = cst32_sb[:, 3 : 3 + DK]
        wv_sb = cst16_sb[:, 0 : NCHUNK * DK]
        mask_sb = cst16_sb[:, NCHUNK * DK :]

        # ---- PE warmup: continuous PE activity from t0 until x8 chunk 0
        # lands (pstate ramps to full clock after ~3us of uninterrupted
        # work) + pulls the exp ACT_TABLE_LOAD early.
        with tc.tile_pool(name="warmps", bufs=1, space="PSUM") as warmps:
            wsrc = sbufs.tile([128, 512], bf16, tag="wsrc")
            nc.vector.memset(wsrc, 0.0)
            wdst = warmps.tile([128, 512], f32, tag="warm")
            for _ in range(WARMUP_MMS):
                nc.tensor.matmul(
                    wdst, lhsT=wsrc[:, 0:128], rhs=wsrc, start=True, stop=True
                )
            wexp = sbufs.tile([128, 1], f32, tag="wexp")
            nc.scalar.activation(out=wexp, in_=wsrc[:, 0:1], func=Act.Exp, scale=1.0)

        # ---- q/k projections, key-chunk-major (fp8 DoubleRow).
        # kT positions 0..7 = own-query columns (x8 chunks 0,1 = qT source),
        # positions 8..15 = chunks 2,3. Each 512-wide psum accumulation
        # completes right after its chunk arrives; per-piece sbuf tiles.
        kts = [sbufs.tile([128, 512], bf16, tag=f"kt{i}", name=f"kt{i}")
               for i in range(4)]
        qT_lo = sbufs.tile([128, 512], bf16, tag="qTl")     # slots 0..3
        qT_hi = sbufs.tile([128, 512], bf16, tag="qTh")     # slots 4..7

        spool = tc.alloc_tile_pool(name="spool", bufs=2, space="PSUM")
        kqpool = tc.alloc_tile_pool(name="kqpool", bufs=2, space="PSUM")

        def kq_mms(dst_ps, w_sb, t):
            for dc in range(NDC):
                nc.tensor.matmul(
                    dst_ps,
                    lhsT=w_sb[:, dc, :, :],
                    rhs=x8s[t][:, dc],
                    start=(dc == 0),
                    stop=(dc == NDC - 1),
                    perf_mode=DR,
                )

        def dve_copy(dst, src_ps, scale, bias):
            nc.vector.tensor_scalar(
                out=dst, in0=src_ps, scalar1=float(scale), scalar2=bias,
                op0=Alu.mult, op1=Alu.add,
            )

        kps = [None] * NKC
        qps = [None] * 2
        # chunk 0: kT piece 0 first (ACT copy), then qT_lo
        kps[0] = kqpool.tile([128, 512], f32, tag="kps", name="kps0")
        kq_mms(kps[0], wk8_sb, 0)
        qps[0] = kqpool.tile([128, 512], f32, tag="qps", name="qps0")
        kq_mms(qps[0], wq8_sb, 0)
        nc.scalar.activation(
            out=kts[0], in_=kps[0], func=Act.Identity, bias=bks_sb,
            scale=SCALE / WS,
        )
        dve_copy(qT_lo, qps[0], 1.0 / WS, bq_sb)
        # chunk 1: qT_hi first (unblocks every exp), then kT piece 1
        qps[1] = kqpool.tile([128, 512], f32, tag="qps", name="qps1")
        kq_mms(qps[1], wq8_sb, 1)
        kps[1] = kqpool.tile([128, 512], f32, tag="kps", name="kps1")
        kq_mms(kps[1], wk8_sb, 1)
        dve_copy(qT_hi, qps[1], 1.0 / WS, bq_sb)
        dve_copy(kts[1], kps[1], SCALE / WS, bks_sb)
        # chunks 2,3: kT pieces 2,3 (psum slots recycled)
        for t in (2, 3):
            kps[t] = kqpool.tile([128, 512], f32, tag="kps", name=f"kps{t}")
            kq_mms(kps[t], wk8_sb, t)
            dve_copy(kts[t], kps[t], SCALE / WS, bks_sb)
        kqpool.release()

        # ---- attention: S^T/exp, v groups, PV bursts ----
        pt_pool = ctx.enter_context(tc.tile_pool(name="pt_pool", bufs=NBLK))
        vpool = tc.alloc_tile_pool(name="vpool", bufs=1, space="PSUM")
        opool = tc.alloc_tile_pool(name="opool", bufs=3, space="PSUM")

        pts = [None] * NBLK
        v_augs = [None] * NG
        o_pss = [None] * NSLOT

        def kpos(j):
            # column position of local key block j in the reordered x8/kT
            return (j - 1) // 2 if j % 2 == 1 else NSLOT + j // 2

        def emit_st(j):
            """S^T for key block j into a 2-bank psum tile, then one exp
            over the whole active range, then the frontier mask multiply."""
            sj = j // 2           # first active slot for this key position
            q0 = 128 * sj
            qn = NSLOT * 128 - q0
            pt = pt_pool.tile([128, qn], bf16, tag="pt", name=f"pt{j}")
            pts[j] = pt
            kp = kpos(j)
            kt = kts[kp // 4]
            kp = kp % 4
            s_ps = spool.tile([128, 1024], f32, tag="st", name=f"s{j}")
            # matmul pieces split at the qT_lo/qT_hi boundary (col 512),
            # which is also the psum bank boundary
            if q0 < 512:
                pieces = [(qT_lo, q0, q0, 512 - q0), (qT_hi, 0, 512, 512)]
            else:
                pieces = [(qT_hi, q0 - 512, q0, 1024 - q0)]
            for qtile, qoff, doff, sz in pieces:
                nc.tensor.matmul(
                    s_ps[:, doff : doff + sz],
                    lhsT=kt[:, 128 * kp : 128 * kp + 128],
                    rhs=qtile[:, qoff : qoff + sz],
                    start=True,
                    stop=True,
                )
            nc.scalar.activation(
                out=pt, in_=s_ps[:, q0:1024], func=Act.Exp, scale=1.0,
            )
            # mask the frontier slot multiplicatively (exp(s+m) = exp(s)*m01):
            # even j -> maskA (wedge-dependent), odd j -> maskB (causal tri)
            sel = j % 2
            nc.vector.tensor_mul(
                pt[:, 0:128],
                pt[:, 0:128],
                mask_sb[:, 128 * sel : 128 * (sel + 1)],
            )

        def emit_vgroup(g):
            """v for key blocks 4g..4g+3, directly in [key, dk] orientation.
            lhsT = x^T key-block (128 keys), rhs = Wv chunk; accumulate over
            the 8 contraction chunks; no bias (folded into the epilogue)."""
            v_aug = const.tile([128, 4, DK + 1], bf16, tag=f"vaug{g}",
                               name=f"vaug{g}")
            v_augs[g] = v_aug
            nc.vector.memset(v_aug[:, :, DK : DK + 1], 1.0)
            vg_ps = vpool.tile([128, 4, DK], f32, tag="vps", name=f"vg{g}")
            for b in range(4):
                for c in range(NCHUNK):
                    nc.tensor.matmul(
                        vg_ps[:, b, :],
                        lhsT=xgs[g][:, c, 128 * b : 128 * (b + 1)],
                        rhs=wv_sb[:, 128 * c : 128 * (c + 1)],
                        start=(c == 0),
                        stop=(c == NCHUNK - 1),
                    )
            nc.vector.tensor_copy(v_aug[:, :, 0:DK], vg_ps)

        def emit_burst(p, j_lo, j_hi):
            """PV accumulation for slot p over key blocks j_lo..j_hi-1."""
            if j_lo == 0:
                o_pss[p] = opool.tile([128, DK + 1], f32, tag="o",
                                      name=f"o_ps{p}")
            o_ps = o_pss[p]
            for jj in range(j_lo, j_hi):
                nc.tensor.matmul(
                    o_ps,
                    lhsT=pts[jj][:, 128 * (p - jj // 2) : 128 * (p - jj // 2) + 128],
                    rhs=v_augs[jj // 4][:, jj % 4, :],
                    start=(jj == 0),
                    stop=(jj == 2 * p + 1),
                )

        def emit_finish(p):
            """out = o_ps * (1/den) + bv, then DMA out."""
            o_ps = o_pss[p]
            rcp = out_pool.tile([128, 1], f32, tag="rcp")
            nc.vector.reciprocal(rcp, o_ps[:, DK : DK + 1])
            ob = out_pool.tile([128, DK], f32, tag="ob")
            nc.vector.scalar_tensor_tensor(
                out=ob, in0=o_ps[:, 0:DK], scalar=rcp, in1=bvv_sb,
                op0=Alu.mult, op1=Alu.add,
            )
            nc.sync.dma_start(out=o[128 * p : 128 * (p + 1), :], in_=ob)

        def full_burst(p):
            emit_burst(p, 0, 2 * p + 2)
            emit_finish(p)

        # emission order = exp order on ACT (the serial resource) with kt
        # readiness (kt0 ~14, kt1 ~15.7, kt2 ~16.5, kt3 ~17.4) and PV-burst
        # unlocking in mind; v groups and bursts fill the PE between S^Ts.
        emit_st(1); emit_st(3); emit_st(5); emit_st(7)
        emit_st(0); emit_st(2)
        emit_vgroup(0)
        emit_st(9); emit_st(11)
        full_burst(0)
        full_burst(1)
        emit_st(4); emit_st(6)
        emit_vgroup(1)
        full_burst(2)
        full_burst(3)
        emit_st(8); emit_st(10)
        emit_vgroup(2)
        emit_st(13); emit_st(15)
        full_burst(4)
        full_burst(5)
        emit_st(12); emit_st(14)
        emit_burst(6, 0, 12)
        emit_burst(7, 0, 12)
        emit_vgroup(3)
        emit_burst(6, 12, 14)
        emit_finish(6)
        emit_burst(7, 12, 16)
        emit_finish(7)

        opool.release()
        vpool.release()
        spool.release()

    nc.compile()
    return nc


def get_built():
    global _built
    if _built is None:
        _built = _build()
    return _built


def _pos2glob(h):
    if h == 0:
        return list(range(NBLK))
    return [j + 1 if j % 2 == 0 else j - 1 for j in range(NBLK)]


def _pack_w_bf16(W):
    """[D, DK] -> [128, NCHUNK*DK] with column block c holding rows 128c..."""
    import ml_dtypes
    return np.ascontiguousarray(
        np.asarray(W, np.float32).reshape(NCHUNK, 128, DK).transpose(1, 0, 2)
        .reshape(128, NCHUNK * DK).astype(ml_dtypes.bfloat16)
    )


def _pack_w_fp8(W):
    """[D, DK] -> [128, NDC*2*DK] e4m3: [p, ((dc*2+i)*DK)+d] = e4m3(WS*W[256dc+128i+p, d])."""
    import ml_dtypes
    Ws = np.asarray(W, np.float32) * WS
    return np.ascontiguousarray(
        Ws.reshape(NDC, 2, 128, DK).transpose(2, 0, 1, 3)
        .reshape(128, NDC * 2 * DK).astype(ml_dtypes.float8_e4m3)
    )


def make_in_map(x_b, Wq, bq, Wk, bk, Wv, bv, h, xT_pre=None, x8T_pre=None):
    """Build one core's input dict. x_b: [T, D] fp32 for this core's batch.
    xT_pre/x8T_pre: optional precomputed transposed/cast copies (shared by
    both wedge cores of a batch; h=0 uses as-is, h=1 column-permutes)."""
    import ml_dtypes
    bf = ml_dtypes.bfloat16
    if xT_pre is None:
        xT_pre = np.ascontiguousarray(x_b.T.astype(bf))
    if x8T_pre is None:
        x8T_pre = np.ascontiguousarray(x_b.T.astype(ml_dtypes.float8_e4m3))
    if h == 0:
        xT_loc, x8T_loc = xT_pre, x8T_pre
    else:
        p2g = _pos2glob(h)
        cols = np.concatenate([np.arange(128 * g, 128 * (g + 1)) for g in p2g])
        xT_loc = np.ascontiguousarray(xT_pre[:, cols])
        x8T_loc = np.ascontiguousarray(x8T_pre[:, cols])
    # x8 column order: own-query blocks (odd locals, slot order) first, then
    # the even locals -- chunks 0,1 feed the q projection and kT pos 0..7
    korder = list(range(1, NBLK, 2)) + list(range(0, NBLK, 2))
    qcols = np.concatenate([np.arange(128 * j, 128 * (j + 1)) for j in korder])
    # x8p[p, ((t*NDC+dc)*2+i)*KCW + c] = x8T[256dc+128i+p, qcols[512t+c]]
    x8q = x8T_loc[:, qcols]                       # [1024 rows d, 2048 cols]
    x8p = np.ascontiguousarray(
        x8q.reshape(NDC, 2, 128, NKC, KCW).transpose(2, 3, 0, 1, 4)
        .reshape(128, NKC * NDC * 2 * KCW)
    )
    # xgp[p, (g*NCHUNK+c)*GW + t'] = xT[128c+p, GW*g+t']  (partition-major)
    xgp = np.ascontiguousarray(
        xT_loc.reshape(NCHUNK, 128, NG, GW).transpose(1, 2, 0, 3)
        .reshape(128, NG * NCHUNK * GW)
    )
    maskA = (np.ones if h == 0 else np.zeros)((128, 128), bf)
    kk = np.arange(128)
    maskB = np.where(kk[:, None] <= kk[None, :], 1.0, 0.0).astype(bf)
    # cst32: [bq, bk*SCALE, pad, bvv(128 cols, bv broadcast to all parts)]
    cst32 = np.zeros((128, 3 + DK), np.float32)
    cst32[:, 0] = np.asarray(bq, np.float32)
    cst32[:, 1] = np.asarray(bk, np.float32) * SCALE
    cst32[:, 3:] = np.asarray(bv, np.float32)[None, :]
    cstf8 = np.concatenate([_pack_w_fp8(Wk), _pack_w_fp8(Wq)], axis=1)
    cst16 = np.concatenate([_pack_w_bf16(Wv), maskA, maskB], axis=1)
    return {
        "x8p": x8p,
        "xgp": xgp,
        "cstf8": np.ascontiguousarray(cstf8),
        "cst32": np.ascontiguousarray(cst32),
        "cst16": np.ascontiguousarray(cst16),
    }


def gather_out(results):
    """results: list of 8 dicts with 'o' [1024, 128] -> full [B, T, DK]."""
    out = np.zeros((B, T, DK), np.float32)
    for core in range(8):
        b, h = core // 2, core % 2
        ob = results[core]["o"]
        for p in range(NSLOT):
            g = 2 * p + 1 - h
            out[b, 128 * g : 128 * (g + 1), :] = ob[128 * p : 128 * (p + 1), :]
    return out


def kernel(x, Wq, bq, Wk, bk, Wv, bv):
    import ml_dtypes
    from concourse.bass_utils import run_bass_kernel_spmd

    x = np.asarray(x, np.float32)
    args = [np.asarray(a, np.float32) for a in (Wq, bq, Wk, bk, Wv, bv)]
    nc = get_built()
    # one transpose+cast per batch, shared by its two wedge cores
    xT_pres = [np.ascontiguousarray(x[b].T.astype(ml_dtypes.bfloat16))
               for b in range(B)]
    x8T_pres = [np.ascontiguousarray(x[b].T.astype(ml_dtypes.float8_e4m3))
                for b in range(B)]
    in_maps = [
        make_in_map(x[core // 2], args[0], args[1], args[2], args[3], args[4],
                    args[5], core % 2, xT_pre=xT_pres[core // 2],
                    x8T_pre=x8T_pres[core // 2])
        for core in range(8)
    ]
    res = run_bass_kernel_spmd(nc, in_maps, core_ids=list(range(8)))
    return gather_out(res.results)


if __name__ == "__main__":
    rng = np.random.default_rng(0)
    x = rng.standard_normal((B, T, D), dtype=np.float32)
    Wq = rng.standard_normal((D, DK), dtype=np.float32) * 0.03
    out = kernel(x, Wq, np.zeros(DK, np.float32), Wq, np.zeros(DK, np.float32),
                 Wq, np.zeros(DK, np.float32))
    print(out.shape)
